# revision 10
# baseline (speedup 1.0000x reference)
"""Multi-head attention (B=4, S=2048, D=768, H=12, Dk=64) on 8 trn2 NeuronCores.

Sharding: 8 cores = 4 batches x 2 head-groups (Megatron-style tensor parallel
over heads within a batch).  Core (b, g) computes, for batch b and its 6 heads:
    Qt = (Wq_g/8) @ q[b].T          [384, 2048]   (transposed layout, dk on partitions)
    Kt = Wk_g @ k[b].T              [384, 2048]
    V  = v[b] @ Wv_g.T              [2048, 384]   (natural layout, with a ones
                                                   column appended per head)
    per head h, per q-tile:
      St  = Kt_h.T @ Qt_h           scores^T tiles  [k, q]
      Et  = exp(St)                 (max-subtraction skipped; |scores| <~ 6)
      Ut  = [V_h | 1].T @ Et        [65, q]  (row 64 = softmax denominator)
      At  = Ut[0:64] * (1/Ut[64])   attention output^T  [64, q]
    outT_partial = Wo[:, g].T.T @ At_all   [768, 2048]
Host sums the two head-group partials per batch, transposes, adds bo.

Matmuls use float32r (full-rate fp32 PE mode); everything else fp32.
"""

import numpy as np

S = 2048          # sequence length
D = 768           # model dim
HG = 6            # heads per group (per core)
DK = 64           # head dim
GP = HG * DK      # group projection width = 384
P = 128           # partitions
QT = 512          # q-tile (matmul moving free dim)
NQT = S // QT     # 4
NKC = S // P      # 16 key chunks
NDC = D // P      # 6 d-chunks
VW = DK + 1       # 65: V columns + ones column

_CACHE = {}


def _build_module(debug=False):
    import concourse.bass as bass
    import concourse.mybir as mybir
    import concourse.tile as tile
    from concourse import bacc

    fp32 = mybir.dt.float32
    fr = mybir.dt.float32r
    EXP = mybir.ActivationFunctionType.Exp

    nc = bacc.Bacc("TRN2", target_bir_lowering=False, debug=False, num_devices=8)

    xqT_d = nc.dram_tensor("xqT", [D, S], fr, kind="ExternalInput")
    xkT_d = nc.dram_tensor("xkT", [D, S], fr, kind="ExternalInput")
    xvT_d = nc.dram_tensor("xvT", [D, S], fr, kind="ExternalInput")
    wqT_d = nc.dram_tensor("wqT", [D, GP], fr, kind="ExternalInput")
    wkT_d = nc.dram_tensor("wkT", [D, GP], fr, kind="ExternalInput")
    wvT_d = nc.dram_tensor("wvT", [D, GP], fr, kind="ExternalInput")
    woT_d = nc.dram_tensor("woT", [GP, D], fr, kind="ExternalInput")
    bq_d = nc.dram_tensor("bq", [1, GP], fr, kind="ExternalInput")
    bk_d = nc.dram_tensor("bk", [1, GP], fr, kind="ExternalInput")
    bv_d = nc.dram_tensor("bv", [1, GP], fr, kind="ExternalInput")
    ones_d = nc.dram_tensor("ones_c", [P, QT], fr, kind="ExternalInput")
    onesv_d = nc.dram_tensor("ones_v", [P, NKC, HG, 1], fr, kind="ExternalInput")
    outT_d = nc.dram_tensor("outT", [D, S], fp32, kind="ExternalOutput")
    if debug:
        dbg_qt = nc.dram_tensor("dbg_qt", [P, GP // P, S], fr, kind="ExternalOutput")
        dbg_kt = nc.dram_tensor("dbg_kt", [P, GP // P, S], fr, kind="ExternalOutput")
        dbg_v = nc.dram_tensor("dbg_v", [P, NKC, HG * VW], fr, kind="ExternalOutput")
        dbg_at = nc.dram_tensor("dbg_at", [P, GP // P, QT], fr, kind="ExternalOutput")
        dbg_et = nc.dram_tensor("dbg_et", [P, 2 * QT], fr, kind="ExternalOutput")
        dbg_ut = nc.dram_tensor("dbg_ut", [VW, 2 * QT], fp32, kind="ExternalOutput")
        dbg_rc = nc.dram_tensor("dbg_rc", [VW, 2, QT], fp32, kind="ExternalOutput")

    with tile.TileContext(nc) as tc:
        with (
            tc.tile_pool(name="persist", bufs=1) as persist,
            tc.tile_pool(name="xio", bufs=2) as xio,
            tc.tile_pool(name="etp", bufs=3) as etp,
            tc.tile_pool(name="small", bufs=1) as small,
            tc.tile_pool(name="tmpp", bufs=2) as tmpp,
            tc.tile_pool(name="outp", bufs=2) as outp,
            tc.tile_pool(name="atp", bufs=2) as atp,
            tc.tile_pool(name="stp", bufs=2, space="PSUM") as stp,
            tc.tile_pool(name="utp", bufs=2, space="PSUM") as utp,
            tc.tile_pool(name="pop", bufs=2, space="PSUM") as pop,
        ):
            # ---- constants & weights -------------------------------------
            ones = persist.tile([P, QT], fr)
            nc.sync.dma_start(ones, ones_d[:])

            wq_sb = persist.tile([P, NDC, GP], fr)
            nc.sync.dma_start(wq_sb, wqT_d[:].rearrange("(c p) m -> p c m", p=P))
            wk_sb = persist.tile([P, NDC, GP], fr)
            nc.sync.dma_start(wk_sb, wkT_d[:].rearrange("(c p) m -> p c m", p=P))
            wv_sb = persist.tile([P, NDC, GP], fr)
            nc.sync.dma_start(wv_sb, wvT_d[:].rearrange("(c p) m -> p c m", p=P))
            wo_sb = persist.tile([P, GP // P, D], fr)
            nc.sync.dma_start(wo_sb, woT_d[:].rearrange("(c p) m -> p c m", p=P))
            bq_sb = persist.tile([1, GP], fr)
            nc.sync.dma_start(bq_sb, bq_d[:])
            bk_sb = persist.tile([1, GP], fr)
            nc.sync.dma_start(bk_sb, bk_d[:])
            bv_sb = persist.tile([1, GP], fr)
            nc.sync.dma_start(bv_sb, bv_d[:])

            # V in natural layout, 65 columns per head (65th = 1.0)
            v_sb = persist.tile([P, NKC, HG * VW], fr)
            nc.sync.dma_start(
                v_sb[:].rearrange("p c (h e) -> p c h e", e=VW)[:, :, :, DK:VW],
                onesv_d[:],
            )

            qt_sb = persist.tile([P, GP // P, S], fr)  # Qt, chunked by dk
            kt_sb = persist.tile([P, GP // P, S], fr)

            # ---- Q/K projections (transposed layout) ---------------------
            for qt in range(NQT):
                qs = slice(qt * QT, (qt + 1) * QT)
                for name, xd, w_sb, b_sb, dst in (
                    ("q", xqT_d, wq_sb, bq_sb, qt_sb),
                    ("k", xkT_d, wk_sb, bk_sb, kt_sb),
                ):
                    x_t = xio.tile([P, NDC, QT], fr, tag="xq")
                    nc.sync.dma_start(
                        x_t, xd[:, qs].rearrange("(c p) q -> p c q", p=P)
                    )
                    for m in range(GP // P):
                        ms = slice(m * P, (m + 1) * P)
                        ps = stp.tile([P, 2 * QT], mybir.dt.float32, tag="st", name="ps_qk")[:, :QT]
                        # bias broadcast along q via K=1 matmul
                        nc.tensor.matmul(
                            ps, lhsT=b_sb[0:1, ms], rhs=ones[0:1, :],
                            start=True, stop=False,
                        )
                        for c in range(NDC):
                            nc.tensor.matmul(
                                ps,
                                lhsT=w_sb[:, c, ms],
                                rhs=x_t[:, c, :],
                                start=False, stop=(c == NDC - 1),
                            )
                        nc.vector.tensor_copy(out=dst[:, m, qs], in_=ps)

            # ---- V projection (natural layout) ---------------------------
            for si in range(NKC):
                ss = slice(si * P, (si + 1) * P)
                xv_t = xio.tile([P, NDC, P], fr, tag="xv")
                nc.sync.dma_start(
                    xv_t, xvT_d[:, ss].rearrange("(c p) s -> p c s", p=P)
                )
                ps = stp.tile([P, 2 * QT], mybir.dt.float32, tag="st", name="ps_v")[:, :GP]
                nc.tensor.matmul(
                    ps, lhsT=ones[0:1, 0:P], rhs=bv_sb[0:1, :],
                    start=True, stop=False,
                )
                for c in range(NDC):
                    nc.tensor.matmul(
                        ps,
                        lhsT=xv_t[:, c, :],
                        rhs=wv_sb[:, c, :],
                        start=False, stop=(c == NDC - 1),
                    )
                nc.vector.tensor_copy(
                    out=v_sb[:, si, :].rearrange("p (h e) -> p h e", e=VW)[:, :, 0:DK],
                    in_=ps.rearrange("p (h d) -> p h d", d=DK),
                )

            # ---- attention + output projection, per q-tile ---------------
            for qt in range(NQT):
                qs = slice(qt * QT, (qt + 1) * QT)
                at_t = atp.tile([P, GP // P, QT], fr, tag="at")
                for p in range(GP // P):  # head pair
                    ut0 = utp.tile([VW, QT], mybir.dt.float32, tag="ut")
                    ut1 = utp.tile([VW, QT], mybir.dt.float32, tag="ut")
                    for kc in range(NKC):
                        ks = slice(kc * P, (kc + 1) * P)
                        st = stp.tile([P, 2 * QT], mybir.dt.float32, tag="st")
                        # two heads packed into PE row groups 0-63 / 64-127
                        nc.tensor.matmul(
                            st[:, 0:QT],
                            lhsT=kt_sb[0:DK, p, ks],
                            rhs=qt_sb[0:DK, p, qs],
                            start=True, stop=True,
                        )
                        nc.tensor.matmul(
                            st[:, QT:],
                            lhsT=kt_sb[DK:P, p, ks],
                            rhs=qt_sb[DK:P, p, qs],
                            start=True, stop=True,
                        )
                        et = etp.tile([P, 2 * QT], fr, tag="et")
                        nc.scalar.activation(out=et, in_=st, func=EXP)
                        if debug and qt == 0 and p == 0 and kc == 0:
                            nc.sync.dma_start(out=dbg_et[:], in_=et[:])
                        nc.tensor.matmul(
                            ut0,
                            lhsT=v_sb[:, kc, 2 * VW * p : 2 * VW * p + VW],
                            rhs=et[:, 0:QT],
                            start=(kc == 0), stop=(kc == NKC - 1),
                        )
                        nc.tensor.matmul(
                            ut1,
                            lhsT=v_sb[:, kc, 2 * VW * p + VW : 2 * VW * (p + 1)],
                            rhs=et[:, QT:],
                            start=(kc == 0), stop=(kc == NKC - 1),
                        )
                    # reciprocal of the denominators (row 64 of ut0/ut1),
                    # computed in-lane on partition 64
                    if debug and qt == 0 and p == 0:
                        utc = outp.tile([VW, 2 * QT], fp32, tag="utc")
                        nc.vector.tensor_copy(out=utc[:, 0:QT], in_=ut0[:])
                        nc.vector.tensor_copy(out=utc[:, QT:], in_=ut1[:])
                        nc.sync.dma_start(out=dbg_ut[:], in_=utc[:])
                    rc = small.tile([VW, 2, QT], fp32, tag="rc")
                    rcr = small.tile([VW, 2, QT], fr, tag="rcr")
                    nc.vector.reciprocal(out=rc[DK:VW, 0, :], in_=ut0[DK:VW, :])
                    nc.vector.reciprocal(out=rc[DK:VW, 1, :], in_=ut1[DK:VW, :])
                    # broadcast 1/denom across 64 partitions via K=1 matmul
                    nc.vector.tensor_copy(out=rcr[DK:VW, :, :], in_=rc[DK:VW, :, :])
                    if debug and qt == 0 and p == 0:
                        nc.sync.dma_start(out=dbg_rc[:], in_=rc[:])
                    bc0 = pop.tile([P, QT], mybir.dt.float32, tag="po", name="bc0")[0:DK, :]
                    nc.tensor.matmul(
                        bc0, lhsT=ones[DK:VW, 0:DK], rhs=rcr[DK:VW, 0, :],
                        start=True, stop=True,
                    )
                    bc1 = pop.tile([P, QT], mybir.dt.float32, tag="po", name="bc1")[0:DK, :]
                    nc.tensor.matmul(
                        bc1, lhsT=ones[DK:VW, 0:DK], rhs=rcr[DK:VW, 1, :],
                        start=True, stop=True,
                    )
                    bc_sb = etp.tile([P, 2 * QT], fp32, tag="et")
                    nc.vector.tensor_copy(out=bc_sb[0:DK, 0:QT], in_=bc0)
                    nc.vector.tensor_copy(out=bc_sb[0:DK, QT:], in_=bc1)
                    # At = Ut * (1/denom)
                    nc.vector.tensor_mul(
                        out=at_t[0:DK, p, :], in0=ut0[0:DK, :], in1=bc_sb[0:DK, 0:QT]
                    )
                    sh = tmpp.tile([DK, QT], fr, tag="sh")
                    nc.vector.tensor_mul(
                        out=sh, in0=ut1[0:DK, :], in1=bc_sb[0:DK, QT:]
                    )
                    # odd head lands on partitions 64-127 via SBUF->SBUF DMA
                    nc.sync.dma_start(out=at_t[DK:P, p, :], in_=sh)

                if debug and qt == 0:
                    nc.sync.dma_start(out=dbg_at[:], in_=at_t[:])
                # ---- output projection for this q-tile -------------------
                for oc in range(NDC):
                    os_ = slice(oc * P, (oc + 1) * P)
                    po = pop.tile([P, QT], mybir.dt.float32, tag="po")
                    for c in range(GP // P):
                        nc.tensor.matmul(
                            po,
                            lhsT=wo_sb[:, c, os_],
                            rhs=at_t[:, c, :],
                            start=(c == 0), stop=(c == GP // P - 1),
                        )
                    ot = outp.tile([P, QT], fp32, tag="ot")
                    nc.vector.tensor_copy(out=ot, in_=po)
                    nc.sync.dma_start(out=outT_d[os_, qs], in_=ot)

            if debug:
                nc.sync.dma_start(out=dbg_qt[:], in_=qt_sb[:])
                nc.sync.dma_start(out=dbg_kt[:], in_=kt_sb[:])
                nc.sync.dma_start(out=dbg_v[:], in_=v_sb[:])
    nc.compile()
    return nc


def _get_module(debug=False):
    key = ("nc", debug)
    if key not in _CACHE:
        _CACHE[key] = _build_module(debug)
    return _CACHE[key]


def make_in_maps(q, k, v, Wq, bq, Wk, bk, Wv, bv, Wo, bo):
    """Shard the full inputs into the 8 per-core input maps."""
    f32 = np.float32
    q = np.asarray(q, f32)
    k = np.asarray(k, f32)
    v = np.asarray(v, f32)
    Wq = np.asarray(Wq, f32)
    Wk = np.asarray(Wk, f32)
    Wv = np.asarray(Wv, f32)
    Wo = np.asarray(Wo, f32)
    bq = np.asarray(bq, f32)
    bk = np.asarray(bk, f32)
    bv = np.asarray(bv, f32)
    B = q.shape[0]
    scale = f32(1.0 / np.sqrt(DK))
    in_maps = []
    for core in range(2 * B):
        b, g = core // 2, core % 2
        hs = slice(GP * g, GP * (g + 1))
        in_maps.append(
            {
                "xqT": np.ascontiguousarray(q[b].T),
                "xkT": np.ascontiguousarray(k[b].T),
                "xvT": np.ascontiguousarray(v[b].T),
                "wqT": np.ascontiguousarray((Wq[hs, :] * scale).T),
                "wkT": np.ascontiguousarray(Wk[hs, :].T),
                "wvT": np.ascontiguousarray(Wv[hs, :].T),
                "woT": np.ascontiguousarray(Wo[:, hs].T),
                "bq": (bq[hs] * scale).reshape(1, GP).copy(),
                "bk": bk[hs].reshape(1, GP).copy(),
                "bv": bv[hs].reshape(1, GP).copy(),
                "ones_c": np.ones((P, QT), f32),
                "ones_v": np.ones((P, NKC, HG, 1), f32),
            }
        )
    return in_maps


def gather_output(results, bo, B=4):
    bo = np.asarray(bo, np.float32)
    out = np.empty((B, S, D), np.float32)
    for b in range(B):
        acc = results[2 * b]["outT"] + results[2 * b + 1]["outT"]
        out[b] = acc.T + bo
    return out


def run(inputs, trace=False, debug=False):
    """Run the kernel; returns (output, BassKernelResults)."""
    import concourse.bass_utils as bass_utils

    nc = _get_module(debug)
    in_maps = make_in_maps(**inputs)
    res = bass_utils.run_bass_kernel_spmd(
        nc, in_maps, core_ids=list(range(8)), trace=trace,
        trace_cores=[0] if trace else None,
    )
    out = gather_output(res.results, inputs["bo"])
    return out, res


def kernel(**inputs) -> np.ndarray:
    out, _ = run(inputs, trace=False)
    return out


# revision 11
# speedup vs baseline: 1.4831x; 1.4831x over previous
"""Multi-head attention (B=4, S=2048, D=768, H=12, Dk=64) on 8 trn2 NeuronCores.

Sharding: 8 cores = 4 batches x 2 head-groups (Megatron-style tensor parallel
over heads within a batch).  Core (b, g) computes, for batch b and its 6 heads:
    Qt = (Wq_g/8) @ q[b].T + bq/8   [384, 2048]   (transposed layout, dk on partitions)
    Kt = Wk_g @ k[b].T + bk         [384, 2048]
    V  = v[b] @ Wv_g.T + bv         [2048, 384]   (natural layout, with a ones
                                                   column appended per head)
    per head h, per q-tile:
      St  = Kt_h.T @ Qt_h           scores^T tiles  [k, q]
      Et  = exp(St)                 (max-subtraction skipped; |scores| <~ 6)
      Ut  = [V_h | 1].T @ Et        [65, q]  (row 64 = softmax denominator)
      At  = Ut[0:64] * (1/Ut[64])   attention output^T  [64, q]
    outT_partial = Wo[:, g]^T-proj of At_all   [768, 2048]
Host sums the two head-group partials per batch, transposes, adds bo.

Matmul operands are fp16 (full-rate on the PE, fp32 PSUM accumulation);
score/Ut accumulators and the normalization stay fp32.
"""

import numpy as np

S = 2048          # sequence length
D = 768           # model dim
HG = 6            # heads per group (per core)
DK = 64           # head dim
GP = HG * DK      # group projection width = 384
P = 128           # partitions
QT = 512          # q-tile (matmul moving free dim)
NQT = S // QT     # 4
NKC = S // P      # 16 key chunks
NDC = D // P      # 6 d-chunks
NMC = GP // P     # 3 dk-chunks (head pairs)
VW = DK + 1       # 65: V columns + ones column

_CACHE = {}


def _build_module(debug=False):
    import concourse.mybir as mybir
    import concourse.tile as tile
    from concourse import bacc

    fp32 = mybir.dt.float32
    fh = mybir.dt.float16
    EXP = mybir.ActivationFunctionType.Exp

    nc = bacc.Bacc("TRN2", target_bir_lowering=False, debug=False, num_devices=8)

    xqT_d = nc.dram_tensor("xqT", [D, S], fh, kind="ExternalInput")
    xkT_d = nc.dram_tensor("xkT", [D, S], fh, kind="ExternalInput")
    xvT_d = nc.dram_tensor("xvT", [D, S], fh, kind="ExternalInput")
    wqT_d = nc.dram_tensor("wqT", [D, GP], fh, kind="ExternalInput")
    wkT_d = nc.dram_tensor("wkT", [D, GP], fh, kind="ExternalInput")
    wvT_d = nc.dram_tensor("wvT", [D, GP], fh, kind="ExternalInput")
    woT_d = nc.dram_tensor("woT", [GP, D], fh, kind="ExternalInput")
    bqp_d = nc.dram_tensor("bqp", [P, NMC], fp32, kind="ExternalInput")
    bkp_d = nc.dram_tensor("bkp", [P, NMC], fp32, kind="ExternalInput")
    bv_d = nc.dram_tensor("bv", [1, GP], fh, kind="ExternalInput")
    ones_d = nc.dram_tensor("ones_c", [P, QT], fh, kind="ExternalInput")
    onesv_d = nc.dram_tensor("ones_v", [P, NKC, HG, 1], fh, kind="ExternalInput")
    sel_d = nc.dram_tensor("sel", [HG, HG * DK], fh, kind="ExternalInput")
    outT_d = nc.dram_tensor("outT", [D, S], fp32, kind="ExternalOutput")
    if debug:
        dbg_qt = nc.dram_tensor("dbg_qt", [P, NMC, S], fh, kind="ExternalOutput")
        dbg_kt = nc.dram_tensor("dbg_kt", [P, NMC, S], fh, kind="ExternalOutput")
        dbg_v = nc.dram_tensor("dbg_v", [P, NKC, HG * VW], fh, kind="ExternalOutput")
        dbg_at = nc.dram_tensor("dbg_at", [P, NMC, QT], fh, kind="ExternalOutput")
        dbg_et = nc.dram_tensor("dbg_et", [P, 2 * QT], fh, kind="ExternalOutput")
        dbg_ut = nc.dram_tensor("dbg_ut", [VW, HG, QT], fh, kind="ExternalOutput")
        dbg_rc = nc.dram_tensor("dbg_rc", [HG, QT], fp32, kind="ExternalOutput")

    with tile.TileContext(nc) as tc:
        with (
            tc.tile_pool(name="persist", bufs=1) as persist,
            tc.tile_pool(name="xio", bufs=3) as xio,
            tc.tile_pool(name="etp", bufs=4) as etp,
            tc.tile_pool(name="small", bufs=2) as small,
            tc.tile_pool(name="tmpp", bufs=2) as tmpp,
            tc.tile_pool(name="outp", bufs=3) as outp,
            tc.tile_pool(name="atp", bufs=2) as atp,
            tc.tile_pool(name="utcp", bufs=2) as utcp,
            tc.tile_pool(name="stp", bufs=2, space="PSUM") as stp,
            tc.tile_pool(name="utp", bufs=2, space="PSUM") as utp,
            tc.tile_pool(name="pop", bufs=2, space="PSUM") as pop,
        ):
            # ---- constants & weights -------------------------------------
            ones = persist.tile([P, QT], fh)
            nc.sync.dma_start(ones, ones_d[:])
            sel_sb = persist.tile([HG, HG * DK], fh)
            nc.sync.dma_start(sel_sb, sel_d[:])

            wq_sb = persist.tile([P, NDC, GP], fh)
            nc.sync.dma_start(wq_sb, wqT_d[:].rearrange("(c p) m -> p c m", p=P))
            wk_sb = persist.tile([P, NDC, GP], fh)
            nc.sync.dma_start(wk_sb, wkT_d[:].rearrange("(c p) m -> p c m", p=P))
            wv_sb = persist.tile([P, NDC, GP], fh)
            nc.sync.dma_start(wv_sb, wvT_d[:].rearrange("(c p) m -> p c m", p=P))
            wo_sb = persist.tile([P, NMC, D], fh)
            nc.sync.dma_start(wo_sb, woT_d[:].rearrange("(c p) m -> p c m", p=P))
            bqp_sb = persist.tile([P, NMC], fp32)
            nc.sync.dma_start(bqp_sb, bqp_d[:])
            bkp_sb = persist.tile([P, NMC], fp32)
            nc.sync.dma_start(bkp_sb, bkp_d[:])
            bv_sb = persist.tile([1, GP], fh)
            nc.sync.dma_start(bv_sb, bv_d[:])

            # V in natural layout, 65 columns per head (65th = 1.0)
            v_sb = persist.tile([P, NKC, HG * VW], fh)
            nc.sync.dma_start(
                v_sb[:].rearrange("p c (h e) -> p c h e", e=VW)[:, :, :, DK:VW],
                onesv_d[:],
            )

            qt_sb = persist.tile([P, NMC, S], fh)  # Qt, chunked by dk
            kt_sb = persist.tile([P, NMC, S], fh)

            # ---- Q/K projections (transposed layout) ---------------------
            for qt in range(NQT):
                qs = slice(qt * QT, (qt + 1) * QT)
                for xd, w_sb, b_sb, dst in (
                    (xqT_d, wq_sb, bqp_sb, qt_sb),
                    (xkT_d, wk_sb, bkp_sb, kt_sb),
                ):
                    x_t = xio.tile([P, NDC, QT], fh, tag="xq")
                    nc.sync.dma_start(
                        x_t, xd[:, qs].rearrange("(c p) q -> p c q", p=P)
                    )
                    for m in range(NMC):
                        ms = slice(m * P, (m + 1) * P)
                        ps = stp.tile(
                            [P, 2 * QT], fp32, tag="st", name="ps_qk"
                        )[:, :QT]
                        for c in range(NDC):
                            nc.tensor.matmul(
                                ps,
                                lhsT=w_sb[:, c, ms],
                                rhs=x_t[:, c, :],
                                start=(c == 0), stop=(c == NDC - 1),
                            )
                        # cast + bias-add (bias varies along partitions)
                        nc.vector.tensor_scalar_add(
                            out=dst[:, m, qs], in0=ps, scalar1=b_sb[:, m : m + 1]
                        )

            # ---- V projection (natural layout) ---------------------------
            for si in range(NKC):
                ss = slice(si * P, (si + 1) * P)
                xv_t = xio.tile([P, NDC, P], fh, tag="xv")
                nc.sync.dma_start(
                    xv_t, xvT_d[:, ss].rearrange("(c p) s -> p c s", p=P)
                )
                ps = stp.tile([P, 2 * QT], fp32, tag="st", name="ps_v")[:, :GP]
                nc.tensor.matmul(
                    ps, lhsT=ones[0:1, 0:P], rhs=bv_sb[0:1, :],
                    start=True, stop=False,
                )
                for c in range(NDC):
                    nc.tensor.matmul(
                        ps,
                        lhsT=xv_t[:, c, :],
                        rhs=wv_sb[:, c, :],
                        start=False, stop=(c == NDC - 1),
                    )
                nc.vector.tensor_copy(
                    out=v_sb[:, si, :].rearrange("p (h e) -> p h e", e=VW)[:, :, 0:DK],
                    in_=ps.rearrange("p (h d) -> p h d", d=DK),
                )

            # ---- attention + output projection, per q-tile ---------------
            for qt in range(NQT):
                qs = slice(qt * QT, (qt + 1) * QT)
                at_t = atp.tile([P, NMC, QT], fh, tag="at")
                utc = utcp.tile([VW, HG, QT], fh, tag="utc")
                for p in range(NMC):  # head pair
                    ut0 = utp.tile([VW, QT], fp32, tag="ut")
                    ut1 = utp.tile([VW, QT], fp32, tag="ut")
                    for kc in range(NKC):
                        ks = slice(kc * P, (kc + 1) * P)
                        st = stp.tile([P, 2 * QT], fp32, tag="st")
                        # two heads packed into PE row groups 0-63 / 64-127
                        nc.tensor.matmul(
                            st[:, 0:QT],
                            lhsT=kt_sb[0:DK, p, ks],
                            rhs=qt_sb[0:DK, p, qs],
                            start=True, stop=True,
                        )
                        nc.tensor.matmul(
                            st[:, QT:],
                            lhsT=kt_sb[DK:P, p, ks],
                            rhs=qt_sb[DK:P, p, qs],
                            start=True, stop=True,
                        )
                        et = etp.tile([P, 2 * QT], fh, tag="et")
                        nc.scalar.activation(out=et, in_=st, func=EXP)
                        if debug and qt == 0 and p == 0 and kc == 0:
                            nc.sync.dma_start(out=dbg_et[:], in_=et[:])
                        nc.tensor.matmul(
                            ut0,
                            lhsT=v_sb[:, kc, 2 * VW * p : 2 * VW * p + VW],
                            rhs=et[:, 0:QT],
                            start=(kc == 0), stop=(kc == NKC - 1),
                        )
                        nc.tensor.matmul(
                            ut1,
                            lhsT=v_sb[:, kc, 2 * VW * p + VW : 2 * VW * (p + 1)],
                            rhs=et[:, QT:],
                            start=(kc == 0), stop=(kc == NKC - 1),
                        )
                    # park Ut in SBUF to release the PSUM accumulators
                    nc.vector.tensor_copy(out=utc[:, 2 * p, :], in_=ut0)
                    nc.vector.tensor_copy(out=utc[:, 2 * p + 1, :], in_=ut1)

                # gather the 6 denominators onto 6 partitions (cross-partition
                # move, so via DMA), then one batched reciprocal
                dn = small.tile([HG, QT], fh, tag="dn")
                for j in range(HG):
                    nc.sync.dma_start(out=dn[j : j + 1, :], in_=utc[DK:VW, j, :])
                rc = small.tile([HG, QT], fp32, tag="rc")
                nc.vector.reciprocal(out=rc, in_=dn)
                rcr = small.tile([HG, QT], fh, tag="rcr")
                nc.vector.tensor_copy(out=rcr, in_=rc)
                if debug and qt == 0:
                    nc.sync.dma_start(out=dbg_rc[:], in_=rc[:])
                    nc.sync.dma_start(out=dbg_ut[:], in_=utc[:])

                for p in range(NMC):
                    # broadcast 1/denom across 64 partitions via K=6 matmul
                    bcE = pop.tile([P, QT], fp32, tag="po", name="bcE")[0:DK, :]
                    nc.tensor.matmul(
                        bcE, lhsT=sel_sb[:, DK * 2 * p : DK * (2 * p + 1)],
                        rhs=rcr, start=True, stop=True,
                    )
                    bcO = pop.tile([P, QT], fp32, tag="po", name="bcO")[0:DK, :]
                    nc.tensor.matmul(
                        bcO, lhsT=sel_sb[:, DK * (2 * p + 1) : DK * (2 * p + 2)],
                        rhs=rcr, start=True, stop=True,
                    )
                    # At = Ut * (1/denom)
                    nc.vector.tensor_mul(
                        out=at_t[0:DK, p, :], in0=utc[0:DK, 2 * p, :], in1=bcE
                    )
                    sh = tmpp.tile([DK, QT], fh, tag="sh")
                    nc.vector.tensor_mul(
                        out=sh, in0=utc[0:DK, 2 * p + 1, :], in1=bcO
                    )
                    # odd head lands on partitions 64-127 via SBUF->SBUF DMA
                    nc.sync.dma_start(out=at_t[DK:P, p, :], in_=sh)

                if debug and qt == 0:
                    nc.sync.dma_start(out=dbg_at[:], in_=at_t[:])

                # ---- output projection for this q-tile -------------------
                for oc in range(NDC):
                    os_ = slice(oc * P, (oc + 1) * P)
                    po = pop.tile([P, QT], fp32, tag="po", name="po")
                    for c in range(NMC):
                        nc.tensor.matmul(
                            po,
                            lhsT=wo_sb[:, c, os_],
                            rhs=at_t[:, c, :],
                            start=(c == 0), stop=(c == NMC - 1),
                        )
                    ot = outp.tile([P, QT], fp32, tag="ot")
                    nc.vector.tensor_copy(out=ot, in_=po)
                    nc.sync.dma_start(out=outT_d[os_, qs], in_=ot)

            if debug:
                nc.sync.dma_start(out=dbg_qt[:], in_=qt_sb[:])
                nc.sync.dma_start(out=dbg_kt[:], in_=kt_sb[:])
                nc.sync.dma_start(out=dbg_v[:], in_=v_sb[:])
    nc.compile()
    return nc


def _get_module(debug=False):
    key = ("nc", debug)
    if key not in _CACHE:
        _CACHE[key] = _build_module(debug)
    return _CACHE[key]


def make_in_maps(q, k, v, Wq, bq, Wk, bk, Wv, bv, Wo, bo):
    """Shard the full inputs into the 8 per-core input maps."""
    f32, f16 = np.float32, np.float16
    q = np.asarray(q, f32)
    k = np.asarray(k, f32)
    v = np.asarray(v, f32)
    Wq = np.asarray(Wq, f32)
    Wk = np.asarray(Wk, f32)
    Wv = np.asarray(Wv, f32)
    Wo = np.asarray(Wo, f32)
    bq = np.asarray(bq, f32)
    bk = np.asarray(bk, f32)
    bv = np.asarray(bv, f32)
    B = q.shape[0]
    scale = f32(1.0 / np.sqrt(DK))
    # sel: kron(I6, ones(1,64)) -- column block j broadcasts head j's row
    sel = np.kron(np.eye(HG, dtype=f16), np.ones((1, DK), f16))
    in_maps = []
    for core in range(2 * B):
        b, g = core // 2, core % 2
        hs = slice(GP * g, GP * (g + 1))
        in_maps.append(
            {
                "xqT": np.ascontiguousarray(q[b].T).astype(f16),
                "xkT": np.ascontiguousarray(k[b].T).astype(f16),
                "xvT": np.ascontiguousarray(v[b].T).astype(f16),
                "wqT": np.ascontiguousarray((Wq[hs, :] * scale).T).astype(f16),
                "wkT": np.ascontiguousarray(Wk[hs, :].T).astype(f16),
                "wvT": np.ascontiguousarray(Wv[hs, :].T).astype(f16),
                "woT": np.ascontiguousarray(Wo[:, hs].T).astype(f16),
                "bqp": np.ascontiguousarray(
                    (bq[hs] * scale).reshape(NMC, P).T
                ).astype(f32),
                "bkp": np.ascontiguousarray(bk[hs].reshape(NMC, P).T).astype(f32),
                "bv": bv[hs].reshape(1, GP).astype(f16),
                "ones_c": np.ones((P, QT), f16),
                "ones_v": np.ones((P, NKC, HG, 1), f16),
                "sel": sel,
            }
        )
    return in_maps


def gather_output(results, bo, B=4):
    bo = np.asarray(bo, np.float32)
    out = np.empty((B, S, D), np.float32)
    for b in range(B):
        acc = results[2 * b]["outT"] + results[2 * b + 1]["outT"]
        out[b] = acc.T + bo
    return out


def run(inputs, trace=False, debug=False):
    """Run the kernel; returns (output, BassKernelResults)."""
    import concourse.bass_utils as bass_utils

    nc = _get_module(debug)
    in_maps = make_in_maps(**inputs)
    res = bass_utils.run_bass_kernel_spmd(
        nc, in_maps, core_ids=list(range(8)), trace=trace,
        trace_cores=[0] if trace else None,
    )
    out = gather_output(res.results, inputs["bo"])
    return out, res


def kernel(**inputs) -> np.ndarray:
    out, _ = run(inputs, trace=False)
    return out
